# revision 11
# baseline (speedup 1.0000x reference)
# Cross-attention SDPA kernel for 8 Trainium2 NeuronCores.
#
# reference semantics (per batch b):
#   Q = y @ Wq + bq            [N, 64]
#   K = z @ Wk + bk            [M, 64]
#   V = z @ Wv + bv            [M, 64]
#   O = softmax(Q K^T / 8) V   [N, 64]
# B=4, M=N=4096, D=512.
#
# Sharding: 8 cores = 4 batches x 2 halves of the query (decoder) length.
# Each core gets z^T[b] (full, [512,4096]) and its y^T half ([512,2048]),
# pre-transposed and cast to bf16 on the host, and produces O rows
# [2048, 64] fp32.
#
# On-core dataflow (S^T layout so the softmax reduction rides the matmul):
#   zt/yt        d on partitions, 4 chunks of 128; DMA issue cost (~0.6us per
#                dma_start on the issuing sequencer) is spread over the three
#                DMA-capable engines (sync/scalar HWDGE + gpsimd SWDGE)
#   fused proj   stationary [wk|wv] (even blocks) / [wv|wk] (odd): one pass
#                over z yields K^T and V^T together; [wq|wq] duplicates Q^T
#                across both partition halves for free
#   K^T          kt_blk[j] [128,512]: tiles 8j..8j+3 on partitions 0:64,
#                tiles 8j+4..8j+7 on 64:128 -> 2-way row-packed S matmuls
#   V            V^T transposed tile-wise on the PE (+ones column appended)
#   S^T pair     two concurrent row-group matmuls (tiles 8j+i, 8j+4+i)
#   E            = exp(S^T * 0.125)  (ScalarE, PSUM -> SBUF bf16)
#   O^T[65,n]   += V_aug.T @ E       (row 64 = softmax denominator)
#   O[n,64]      = transpose(O^T) * (1/denom) + bv  -> DRAM
#
# bq/bk fold into the PSUM->SBUF projection copies as per-partition bias; bv
# is added at the end (softmax rows sum to 1 so P @ (1 bv^T) == 1 bv^T).

import numpy as np
import ml_dtypes

B = 4
M = 4096
N = 4096
D = 512
KQ = 64
NH = N // 2          # per-core query rows
NCORES = 8
SCALE = 0.125        # 1/sqrt(64)

NT = M // 128        # 32 m-tiles
NPAIR = NT // 2      # 16 pairs (8j+i, 8j+4+i)
NBLK = 4             # n-blocks of 512 in the attention loop
QBLK = NH // 512     # 4 q-projection blocks
KBLK = M // 512      # 8 fused kv-projection blocks

_CACHE = {}


def build_program():
    from contextlib import ExitStack

    import concourse.bacc as bacc
    import concourse.mybir as mybir
    import concourse.tile as tile
    from concourse.bass import ts, ds
    from concourse.masks import make_identity

    F32 = mybir.dt.float32
    BF16 = mybir.dt.bfloat16
    EXP = mybir.ActivationFunctionType.Exp

    nc = bacc.Bacc("TRN2", target_bir_lowering=False, debug=False)

    zt_d = nc.dram_tensor("zt", [4, 128, M], BF16, kind="ExternalInput").ap()
    yt_d = nc.dram_tensor("yt", [4, 128, NH], BF16, kind="ExternalInput").ap()
    # 12 fused weight tiles: [wq|wq]x4, [wk|wv]x4, [wv|wk]x4
    wp_d = nc.dram_tensor("wpack", [128, 1536], BF16, kind="ExternalInput").ap()
    # col 0: bq (dup both halves), col 1: bk (dup)
    bp_d = nc.dram_tensor("bpack", [128, 2], F32, kind="ExternalInput").ap()
    o_d = nc.dram_tensor("o", [KQ + 1, NH], F32, kind="ExternalOutput").ap()

    with ExitStack() as ctx:
        tc = ctx.enter_context(tile.TileContext(nc))
        singles = ctx.enter_context(tc.tile_pool(name="singles", bufs=1))
        epool = ctx.enter_context(tc.tile_pool(name="epool", bufs=4))
        vtpool = ctx.enter_context(tc.tile_pool(name="vtpool", bufs=2))
        otpool = ctx.enter_context(tc.tile_pool(name="otpool", bufs=2))
        spool = ctx.enter_context(tc.tile_pool(name="spool", bufs=2, space="PSUM"))
        opool = ctx.enter_context(tc.tile_pool(name="opool", bufs=2, space="PSUM"))
        ppool = ctx.enter_context(tc.tile_pool(name="ppool", bufs=2, space="PSUM"))

        # --- constants ---
        wpack = singles.tile([128, 1536], BF16, name="wpack", tag="wpack")
        nc.sync.dma_start(wpack, wp_d)
        bpack = singles.tile([128, 2], F32, name="bpack", tag="bpack")
        nc.sync.dma_start(bpack, bp_d)
        wqq = [wpack[:, ts(c, 128)] for c in range(4)]
        wkv_e = [wpack[:, ds(512 + c * 128, 128)] for c in range(4)]
        wkv_o = [wpack[:, ds(1024 + c * 128, 128)] for c in range(4)]
        bq_sb = bpack[:, 0:1]
        bk_sb = bpack[:, 1:2]

        identb = singles.tile([128, 128], BF16, name="identb", tag="identb")
        make_identity(nc, identb)

        # warm the exp table while DMAs stream
        warm = singles.tile([64, 1], F32, name="warm", tag="warm")
        nc.scalar.activation(warm, bpack[0:64, 0:1], EXP, scale=1.0)

        # --- activation SBUF tensors (filled by the DMAs below) ---
        yt = [
            singles.tile([128, NH], BF16, name=f"yt{c}", tag=f"yt{c}")
            for c in range(4)
        ]
        zt = [
            [
                singles.tile([128, 2048], BF16, name=f"zt{c}{h}", tag=f"zt{c}{h}")
                for h in range(2)
            ]
            for c in range(4)
        ]
        qt_blk = [
            singles.tile([128, 512], BF16, name=f"qt{j}", tag=f"qt{j}")
            for j in range(QBLK)
        ]
        kt_blk = [
            singles.tile([128, 512], BF16, name=f"kt{j}", tag=f"kt{j}")
            for j in range(4)
        ]
        v_sb = [
            singles.tile([128, KQ + 1], BF16, name=f"v{t}", tag=f"v{t}")
            for t in range(NT)
        ]

        # --- all input DMAs issued upfront, spread over 3 issuing engines
        # (each engine owns one DMA queue; ~95-128 GB/s per queue). y piece 0
        # gates q_proj(0) and with it every S matmul, so it goes first on
        # every queue; z blocks follow in consumption order. ---
        engs = [nc.sync, nc.scalar, nc.gpsimd]

        def dma_y_piece(pc):
            for c in range(4):
                engs[c % 3].dma_start(
                    yt[c][:, ds(pc * 1024, 1024)],
                    yt_d[c, :, ds(pc * 1024, 1024)],
                )

        def dma_z_piece(h, pc):
            for c in range(4):
                engs[(c + h + pc) % 3].dma_start(
                    zt[c][h][:, ds(pc * 1024, 1024)],
                    zt_d[c, :, ds(h * 2048 + pc * 1024, 1024)],
                )

        dma_y_piece(0)
        dma_z_piece(0, 0)
        dma_z_piece(0, 1)
        dma_y_piece(1)
        dma_z_piece(1, 0)
        dma_z_piece(1, 1)

        def q_proj(j):
            # [wq|wq] stationary -> Q^T appears on both partition halves
            q_ps = ppool.tile([128, 512], F32, name="proj", tag="proj")
            for c in range(4):
                nc.tensor.matmul(
                    q_ps,
                    lhsT=wqq[c],
                    rhs=yt[c][:, ts(j, 512)],
                    start=(c == 0),
                    stop=(c == 3),
                )
            nc.vector.tensor_scalar_add(qt_blk[j], q_ps, bq_sb)

        def kv_proj(b):
            # one pass over z m-block b (tiles 4b..4b+3) produces
            # K^T -> kt_blk[b//2] (even b: partitions 0:64, odd: 64:128)
            # V^T -> staging, then PE-transposed into v_sb tiles
            h = b // 4
            bb = b % 4
            odd = b % 2
            w = wkv_o if odd else wkv_e
            khalf = slice(64, 128) if odd else slice(0, 64)
            vhalf = slice(0, 64) if odd else slice(64, 128)
            kv_ps = ppool.tile([128, 512], F32, name="proj", tag="proj")
            for c in range(4):
                nc.tensor.matmul(
                    kv_ps,
                    lhsT=w[c],
                    rhs=zt[c][h][:, ts(bb, 512)],
                    start=(c == 0),
                    stop=(c == 3),
                )
            nc.vector.tensor_scalar_add(
                kt_blk[b // 2][khalf, :], kv_ps[khalf, :], bk_sb[khalf, :]
            )
            vt_sb = vtpool.tile([128, 512], BF16, name="vt", tag="vt")
            nc.vector.tensor_copy(vt_sb[vhalf, :], kv_ps[vhalf, :])
            vib = identb[64:128, 64:128] if odd == 0 else identb[0:64, 0:64]
            for i in range(4):
                t = 4 * b + i
                v_ps = ppool.tile([128, 512], BF16, name="projb", tag="proj")
                nc.tensor.matmul(
                    v_ps[:, 0:KQ],
                    lhsT=vt_sb[vhalf, ts(i, 128)],
                    rhs=vib,
                    is_transpose=True,
                    start=True,
                    stop=True,
                    tile_position=(vhalf.start, 0),
                )
                nc.vector.tensor_copy(v_sb[t][:, 0:KQ], v_ps[:, 0:KQ])
                nc.vector.memset(v_sb[t][:, KQ : KQ + 1], 1.0)

        def attn_pair(nb, p, o_ps):
            jj, col = divmod(p, 4)
            qlo = qt_blk[nb][0:64, :]
            qhi = qt_blk[nb][64:128, :]
            s_ps = spool.tile([128, 1024], F32, name="s", tag="s")
            nc.tensor.matmul(
                s_ps[:, 0:512],
                lhsT=kt_blk[jj][0:64, ts(col, 128)],
                rhs=qlo,
                start=True,
                stop=True,
                tile_position=(0, 0),
            )
            nc.tensor.matmul(
                s_ps[:, 512:1024],
                lhsT=kt_blk[jj][64:128, ts(col, 128)],
                rhs=qhi,
                start=True,
                stop=True,
                tile_position=(64, 0),
            )
            e_t = epool.tile([128, 1024], BF16, name="e", tag="e")
            nc.scalar.activation(e_t, s_ps, EXP, scale=SCALE)
            nc.tensor.matmul(
                o_ps[0:65, :],
                lhsT=v_sb[8 * jj + col],
                rhs=e_t[:, 0:512],
                start=(p == 0),
                stop=False,
            )
            nc.tensor.matmul(
                o_ps[0:65, :],
                lhsT=v_sb[8 * jj + 4 + col],
                rhs=e_t[:, 512:1024],
                start=False,
                stop=(p == NPAIR - 1),
            )

        def finalize(nb, o_ps):
            # ship unnormalized O^T (+denominator row 64) out; the host does
            # the cheap per-row divide and bias add during unsharding
            ot_sb = otpool.tile([128, 512], F32, name="ot", tag="ot")
            nc.vector.tensor_copy(ot_sb[0:65, :], o_ps[0:65, :])
            nc.sync.dma_start(o_d[:, ds(nb * 512, 512)], ot_sb[0:65, :])

        # --- interleaved emission: attention pairs ride along as their
        # projection blocks (z m-ranges) become available ---
        q_proj(0)
        o_ps0 = opool.tile([128, 512], F32, name="o", tag="o")
        for grp in range(4):
            kv_proj(2 * grp)
            kv_proj(2 * grp + 1)
            if grp == 2:
                q_proj(1)
            for p in range(4 * grp, 4 * grp + 4):
                attn_pair(0, p, o_ps0)
        finalize(0, o_ps0)
        q_proj(2)
        for nb in range(1, NBLK):
            o_ps = opool.tile([128, 512], F32, name="o", tag="o")
            if nb == 2:
                q_proj(3)
            for p in range(NPAIR):
                attn_pair(nb, p, o_ps)
            finalize(nb, o_ps)

    nc.compile()
    return nc


def _get_program():
    if "nc" not in _CACHE:
        _CACHE["nc"] = build_program()
    return _CACHE["nc"]


def make_in_maps(z, y, Wq, bq, Wk, bk, Wv, bv):
    bf16 = ml_dtypes.bfloat16
    zt = np.ascontiguousarray(z.astype(bf16).transpose(0, 2, 1))  # [B, 512, M]
    yt = np.ascontiguousarray(y.astype(bf16).transpose(0, 2, 1))  # [B, 512, N]
    wq = Wq.astype(bf16).reshape(4, 128, KQ)
    wk = Wk.astype(bf16).reshape(4, 128, KQ)
    wv = Wv.astype(bf16).reshape(4, 128, KQ)
    wpack = np.empty((128, 1536), dtype=bf16)
    for c in range(4):
        wpack[:, c * 128 : c * 128 + 64] = wq[c]
        wpack[:, c * 128 + 64 : c * 128 + 128] = wq[c]
        wpack[:, 512 + c * 128 : 512 + c * 128 + 64] = wk[c]
        wpack[:, 512 + c * 128 + 64 : 512 + c * 128 + 128] = wv[c]
        wpack[:, 1024 + c * 128 : 1024 + c * 128 + 64] = wv[c]
        wpack[:, 1024 + c * 128 + 64 : 1024 + c * 128 + 128] = wk[c]
    bpack = np.empty((128, 2), dtype=np.float32)
    bpack[0:64, 0] = bq
    bpack[64:128, 0] = bq
    bpack[0:64, 1] = bk
    bpack[64:128, 1] = bk
    in_maps = []
    for c in range(NCORES):
        b, h = divmod(c, 2)
        in_maps.append(
            {
                "zt": zt[b].reshape(4, 128, M),
                "yt": np.ascontiguousarray(
                    yt[b][:, h * NH : (h + 1) * NH]
                ).reshape(4, 128, NH),
                "wpack": wpack,
                "bpack": bpack,
            }
        )
    return in_maps


def kernel(z, y, Wq, bq, Wk, bk, Wv, bv):
    from concourse import bass_utils

    nc = _get_program()
    in_maps = make_in_maps(z, y, Wq, bq, Wk, bk, Wv, bv)
    res = bass_utils.run_bass_kernel_spmd(nc, in_maps, core_ids=list(range(NCORES)))
    return assemble_output(res.results, bv)


def assemble_output(results, bv):
    out = np.empty((B, N, KQ), dtype=np.float32)
    bvf = bv.astype(np.float32)[None, :]
    for c in range(NCORES):
        b, h = divmod(c, 2)
        ot = results[c]["o"]  # [65, NH]: rows 0:64 = O^T unnorm, row 64 = denom
        out[b, h * NH : (h + 1) * NH, :] = (ot[0:KQ] / ot[KQ : KQ + 1]).T + bvf
    return out


# revision 12
# speedup vs baseline: 1.0851x; 1.0851x over previous
# Cross-attention SDPA kernel for 8 Trainium2 NeuronCores.
#
# reference semantics (per batch b):
#   Q = y @ Wq + bq            [N, 64]
#   K = z @ Wk + bk            [M, 64]
#   V = z @ Wv + bv            [M, 64]
#   O = softmax(Q K^T / 8) V   [N, 64]
# B=4, M=N=4096, D=512.
#
# Sharding: 8 cores = 4 batches x 2 halves of the query (decoder) length.
# Each core gets z^T[b] (full, [512,4096]) and its y^T half ([512,2048]),
# pre-transposed and cast to bf16 on the host, and produces O rows
# [2048, 64] fp32.
#
# On-core dataflow (S^T layout so the softmax reduction rides the matmul):
#   zt/yt        d on partitions, 4 chunks of 128; DMA issue cost (~0.6us per
#                dma_start on the issuing sequencer) is spread over the three
#                DMA-capable engines (sync/scalar HWDGE + gpsimd SWDGE)
#   fused proj   stationary [wk|wv] (even blocks) / [wv|wk] (odd): one pass
#                over z yields K^T and V^T together; [wq|wq] duplicates Q^T
#                across both partition halves for free
#   K^T          kt_blk[j] [128,512]: tiles 8j..8j+3 on partitions 0:64,
#                tiles 8j+4..8j+7 on 64:128 -> 2-way row-packed S matmuls
#   V            V^T transposed tile-wise on the PE (+ones column appended)
#   S^T pair     two concurrent row-group matmuls (tiles 8j+i, 8j+4+i)
#   E            = exp(S^T * 0.125)  (ScalarE, PSUM -> SBUF bf16)
#   O^T[65,n]   += V_aug.T @ E       (row 64 = softmax denominator)
#   O[n,64]      = transpose(O^T) * (1/denom) + bv  -> DRAM
#
# bq/bk fold into the PSUM->SBUF projection copies as per-partition bias; bv
# is added at the end (softmax rows sum to 1 so P @ (1 bv^T) == 1 bv^T).

import numpy as np
import ml_dtypes

B = 4
M = 4096
N = 4096
D = 512
KQ = 64
NH = N // 2          # per-core query rows
NCORES = 8
SCALE = 0.125        # 1/sqrt(64)

NT = M // 128        # 32 m-tiles
NPAIR = NT // 2      # 16 pairs (8j+i, 8j+4+i)
NBLK = 4             # n-blocks of 512 in the attention loop
QBLK = NH // 512     # 4 q-projection blocks
KBLK = M // 512      # 8 fused kv-projection blocks

_CACHE = {}


def build_program():
    from contextlib import ExitStack

    import concourse.bacc as bacc
    import concourse.mybir as mybir
    import concourse.tile as tile
    from concourse.bass import ts, ds
    from concourse.masks import make_identity

    F32 = mybir.dt.float32
    BF16 = mybir.dt.bfloat16
    EXP = mybir.ActivationFunctionType.Exp

    nc = bacc.Bacc("TRN2", target_bir_lowering=False, debug=False)

    zt_d = nc.dram_tensor("zt", [4, 128, M], BF16, kind="ExternalInput").ap()
    yt_d = nc.dram_tensor("yt", [4, 128, NH], BF16, kind="ExternalInput").ap()
    # 12 fused weight tiles: [wq|wq]x4, [wk|wv]x4, [wv|wk]x4
    wp_d = nc.dram_tensor("wpack", [128, 1536], BF16, kind="ExternalInput").ap()
    # col 0: bq (dup both halves), col 1: bk (dup)
    bp_d = nc.dram_tensor("bpack", [128, 2], F32, kind="ExternalInput").ap()
    o_d = nc.dram_tensor("o", [KQ + 1, NH], F32, kind="ExternalOutput").ap()

    with ExitStack() as ctx:
        tc = ctx.enter_context(tile.TileContext(nc))
        singles = ctx.enter_context(tc.tile_pool(name="singles", bufs=1))
        epool = ctx.enter_context(tc.tile_pool(name="epool", bufs=4))
        vtpool = ctx.enter_context(tc.tile_pool(name="vtpool", bufs=2))
        otpool = ctx.enter_context(tc.tile_pool(name="otpool", bufs=2))
        spool = ctx.enter_context(tc.tile_pool(name="spool", bufs=2, space="PSUM"))
        opool = ctx.enter_context(tc.tile_pool(name="opool", bufs=1, space="PSUM"))
        ppool = ctx.enter_context(tc.tile_pool(name="ppool", bufs=3, space="PSUM"))

        # --- constants ---
        wpack = singles.tile([128, 1536], BF16, name="wpack", tag="wpack")
        nc.sync.dma_start(wpack[:, 0:512], wp_d[:, 0:512])
        nc.scalar.dma_start(wpack[:, 512:1024], wp_d[:, 512:1024])
        nc.gpsimd.dma_start(wpack[:, 1024:1536], wp_d[:, 1024:1536])
        bpack = singles.tile([128, 2], F32, name="bpack", tag="bpack")
        nc.sync.dma_start(bpack, bp_d)
        wqq = [wpack[:, ts(c, 128)] for c in range(4)]
        wkv_e = [wpack[:, ds(512 + c * 128, 128)] for c in range(4)]
        wkv_o = [wpack[:, ds(1024 + c * 128, 128)] for c in range(4)]
        bq_sb = bpack[:, 0:1]
        bk_sb = bpack[:, 1:2]

        identb = singles.tile([128, 128], BF16, name="identb", tag="identb")
        make_identity(nc, identb)

        # warm the exp table while DMAs stream
        warm = singles.tile([64, 1], F32, name="warm", tag="warm")
        nc.scalar.activation(warm, bpack[0:64, 0:1], EXP, scale=1.0)

        # --- activation SBUF tensors (filled by the DMAs below) ---
        yt = [
            singles.tile([128, NH], BF16, name=f"yt{c}", tag=f"yt{c}")
            for c in range(4)
        ]
        zt = [
            [
                singles.tile([128, 2048], BF16, name=f"zt{c}{h}", tag=f"zt{c}{h}")
                for h in range(2)
            ]
            for c in range(4)
        ]
        qt_blk = [
            singles.tile([128, 512], BF16, name=f"qt{j}", tag=f"qt{j}")
            for j in range(QBLK)
        ]
        kt_blk = [
            singles.tile([128, 512], BF16, name=f"kt{j}", tag=f"kt{j}")
            for j in range(4)
        ]
        v_sb = [
            singles.tile([128, KQ + 1], BF16, name=f"v{t}", tag=f"v{t}")
            for t in range(NT)
        ]

        # --- all input DMAs issued upfront, spread over 3 issuing engines
        # (each engine owns one DMA queue; ~95-128 GB/s per queue). y piece 0
        # gates q_proj(0) and with it every S matmul, so it goes first on
        # every queue; z blocks follow in consumption order. ---
        engs = [nc.sync, nc.scalar, nc.gpsimd]

        def dma_y_piece(pc):
            for c in range(4):
                engs[c % 3].dma_start(
                    yt[c][:, ds(pc * 1024, 1024)],
                    yt_d[c, :, ds(pc * 1024, 1024)],
                )

        def dma_z_piece(h, pc):
            for c in range(4):
                engs[(c + h + pc) % 3].dma_start(
                    zt[c][h][:, ds(pc * 1024, 1024)],
                    zt_d[c, :, ds(h * 2048 + pc * 1024, 1024)],
                )

        dma_y_piece(0)
        dma_z_piece(0, 0)
        dma_z_piece(0, 1)
        dma_y_piece(1)
        dma_z_piece(1, 0)
        dma_z_piece(1, 1)

        IDENT_FN = mybir.ActivationFunctionType.Identity

        def q_proj(j, on_act=False):
            # [wq|wq] stationary -> Q^T appears on both partition halves
            q_ps = ppool.tile([128, 512], F32, name="proj", tag="proj")
            for c in range(4):
                nc.tensor.matmul(
                    q_ps,
                    lhsT=wqq[c],
                    rhs=yt[c][:, ts(j, 512)],
                    start=(c == 0),
                    stop=(c == 3),
                )
            if on_act:
                nc.scalar.activation(qt_blk[j], q_ps, IDENT_FN, bias=bq_sb)
            else:
                nc.vector.tensor_scalar_add(qt_blk[j], q_ps, bq_sb)

        def kv_mm(b, on_act=False):
            # one pass over z m-block b (tiles 4b..4b+3) produces
            # K^T -> kt_blk[b//2] (even b: partitions 0:64, odd: 64:128)
            # and V^T -> staging tile (returned for kv_trans)
            h = b // 4
            bb = b % 4
            odd = b % 2
            w = wkv_o if odd else wkv_e
            khalf = slice(64, 128) if odd else slice(0, 64)
            vhalf = slice(0, 64) if odd else slice(64, 128)
            kv_ps = ppool.tile([128, 512], F32, name="proj", tag="proj")
            for c in range(4):
                nc.tensor.matmul(
                    kv_ps,
                    lhsT=w[c],
                    rhs=zt[c][h][:, ts(bb, 512)],
                    start=(c == 0),
                    stop=(c == 3),
                )
            if on_act:
                nc.scalar.activation(
                    kt_blk[b // 2][khalf, :], kv_ps[khalf, :], IDENT_FN,
                    bias=bk_sb[khalf, :],
                )
            else:
                nc.vector.tensor_scalar_add(
                    kt_blk[b // 2][khalf, :], kv_ps[khalf, :], bk_sb[khalf, :]
                )
            vt_sb = vtpool.tile([128, 512], BF16, name="vt", tag="vt")
            nc.vector.tensor_copy(vt_sb[vhalf, :], kv_ps[vhalf, :])
            return vt_sb, vhalf

        def kv_trans(b, vt_sb, vhalf):
            # V^T staging -> PE transpose -> natural-layout V tiles (+ones col)
            odd = b % 2
            vib = identb[64:128, 64:128] if odd == 0 else identb[0:64, 0:64]
            for i in range(4):
                t = 4 * b + i
                v_ps = ppool.tile([128, 512], BF16, name="projb", tag="proj")
                nc.tensor.matmul(
                    v_ps[:, 0:KQ],
                    lhsT=vt_sb[vhalf, ts(i, 128)],
                    rhs=vib,
                    is_transpose=True,
                    start=True,
                    stop=True,
                    tile_position=(vhalf.start, 0),
                )
                nc.vector.tensor_copy(v_sb[t][:, 0:KQ], v_ps[:, 0:KQ])
                nc.vector.memset(v_sb[t][:, KQ : KQ + 1], 1.0)

        def finalize(nb, o_ps):
            # ship unnormalized O^T (+denominator row 64) out; the host does
            # the cheap per-row divide and bias add during unsharding
            ot_sb = otpool.tile([128, 512], F32, name="ot", tag="ot")
            nc.vector.tensor_copy(ot_sb[0:65, :], o_ps[0:65, :])
            nc.sync.dma_start(o_d[:, ds(nb * 512, 512)], ot_sb[0:65, :])

        # --- PE warm-up spin: ~4us of dummy back-to-back matmuls while the
        # first input DMAs stream, so the HAM clock-gate opens (2.4 GHz)
        # before the projection chain begins ---
        wu_ps = opool.tile([128, 512], F32, name="o", tag="o")
        for _ in range(56):
            nc.tensor.matmul(
                wu_ps[:, 0:128],
                lhsT=identb,
                rhs=identb,
                start=True,
                stop=True,
            )

        # --- interleaved emission: attention pairs ride along as their
        # projection blocks (z m-ranges) become available; S matmuls are
        # emitted ahead of the V transposes they do not depend on ---
        q_proj(0, on_act=True)
        o_ps0 = opool.tile([128, 512], F32, name="o", tag="o")
        s_tiles = {}

        def s_mms(nb, p):
            jj, col = divmod(p, 4)
            s_ps = spool.tile([128, 1024], F32, name="s", tag="s")
            nc.tensor.matmul(
                s_ps[:, 0:512],
                lhsT=kt_blk[jj][0:64, ts(col, 128)],
                rhs=qt_blk[nb][0:64, :],
                start=True,
                stop=True,
                tile_position=(0, 0),
            )
            nc.tensor.matmul(
                s_ps[:, 512:1024],
                lhsT=kt_blk[jj][64:128, ts(col, 128)],
                rhs=qt_blk[nb][64:128, :],
                start=True,
                stop=True,
                tile_position=(64, 0),
            )
            s_tiles[(nb, p)] = s_ps

        def exp_pv(nb, p, o_ps):
            jj, col = divmod(p, 4)
            s_ps = s_tiles.pop((nb, p))
            e_t = epool.tile([128, 1024], BF16, name="e", tag="e")
            nc.scalar.activation(e_t, s_ps, EXP, scale=SCALE)
            nc.tensor.matmul(
                o_ps[0:65, :],
                lhsT=v_sb[8 * jj + col],
                rhs=e_t[:, 0:512],
                start=(p == 0),
                stop=False,
            )
            nc.tensor.matmul(
                o_ps[0:65, :],
                lhsT=v_sb[8 * jj + 4 + col],
                rhs=e_t[:, 512:1024],
                start=False,
                stop=(p == NPAIR - 1),
            )

        for grp in range(4):
            vt_a = kv_mm(2 * grp, on_act=(grp == 0))
            vt_b = kv_mm(2 * grp + 1, on_act=(grp == 0))
            s_mms(0, 4 * grp)
            s_mms(0, 4 * grp + 1)
            kv_trans(2 * grp, *vt_a)
            kv_trans(2 * grp + 1, *vt_b)
            exp_pv(0, 4 * grp, o_ps0)
            s_mms(0, 4 * grp + 2)
            exp_pv(0, 4 * grp + 1, o_ps0)
            s_mms(0, 4 * grp + 3)
            exp_pv(0, 4 * grp + 2, o_ps0)
            if grp == 2:
                q_proj(1)
            exp_pv(0, 4 * grp + 3, o_ps0)
        finalize(0, o_ps0)
        q_proj(2)
        for nb in range(1, NBLK):
            o_ps = opool.tile([128, 512], F32, name="o", tag="o")
            if nb == 2:
                q_proj(3)
            for p in range(NPAIR):
                s_mms(nb, p)
                exp_pv(nb, p, o_ps)
            finalize(nb, o_ps)

    nc.compile()
    return nc


def _get_program():
    if "nc" not in _CACHE:
        _CACHE["nc"] = build_program()
    return _CACHE["nc"]


def make_in_maps(z, y, Wq, bq, Wk, bk, Wv, bv):
    bf16 = ml_dtypes.bfloat16
    zt = np.ascontiguousarray(z.astype(bf16).transpose(0, 2, 1))  # [B, 512, M]
    yt = np.ascontiguousarray(y.astype(bf16).transpose(0, 2, 1))  # [B, 512, N]
    wq = Wq.astype(bf16).reshape(4, 128, KQ)
    wk = Wk.astype(bf16).reshape(4, 128, KQ)
    wv = Wv.astype(bf16).reshape(4, 128, KQ)
    wpack = np.empty((128, 1536), dtype=bf16)
    for c in range(4):
        wpack[:, c * 128 : c * 128 + 64] = wq[c]
        wpack[:, c * 128 + 64 : c * 128 + 128] = wq[c]
        wpack[:, 512 + c * 128 : 512 + c * 128 + 64] = wk[c]
        wpack[:, 512 + c * 128 + 64 : 512 + c * 128 + 128] = wv[c]
        wpack[:, 1024 + c * 128 : 1024 + c * 128 + 64] = wv[c]
        wpack[:, 1024 + c * 128 + 64 : 1024 + c * 128 + 128] = wk[c]
    bpack = np.empty((128, 2), dtype=np.float32)
    bpack[0:64, 0] = bq
    bpack[64:128, 0] = bq
    bpack[0:64, 1] = bk
    bpack[64:128, 1] = bk
    in_maps = []
    for c in range(NCORES):
        b, h = divmod(c, 2)
        in_maps.append(
            {
                "zt": zt[b].reshape(4, 128, M),
                "yt": np.ascontiguousarray(
                    yt[b][:, h * NH : (h + 1) * NH]
                ).reshape(4, 128, NH),
                "wpack": wpack,
                "bpack": bpack,
            }
        )
    return in_maps


def kernel(z, y, Wq, bq, Wk, bk, Wv, bv):
    from concourse import bass_utils

    nc = _get_program()
    in_maps = make_in_maps(z, y, Wq, bq, Wk, bk, Wv, bv)
    res = bass_utils.run_bass_kernel_spmd(nc, in_maps, core_ids=list(range(NCORES)))
    return assemble_output(res.results, bv)


def assemble_output(results, bv):
    out = np.empty((B, N, KQ), dtype=np.float32)
    bvf = bv.astype(np.float32)[None, :]
    for c in range(NCORES):
        b, h = divmod(c, 2)
        ot = results[c]["o"]  # [65, NH]: rows 0:64 = O^T unnorm, row 64 = denom
        out[b, h * NH : (h + 1) * NH, :] = (ot[0:KQ] / ot[KQ : KQ + 1]).T + bvf
    return out


# revision 14
# speedup vs baseline: 1.0883x; 1.0029x over previous
# Cross-attention SDPA kernel for 8 Trainium2 NeuronCores.
#
# reference semantics (per batch b):
#   Q = y @ Wq + bq            [N, 64]
#   K = z @ Wk + bk            [M, 64]
#   V = z @ Wv + bv            [M, 64]
#   O = softmax(Q K^T / 8) V   [N, 64]
# B=4, M=N=4096, D=512.
#
# Sharding: 8 cores = 4 batches x 2 halves of the query (decoder) length.
# Each core gets z^T[b] (full, [512,4096]) and its y^T half ([512,2048]),
# pre-transposed and cast to bf16 on the host, and produces O rows
# [2048, 64] fp32.
#
# On-core dataflow (S^T layout so the softmax reduction rides the matmul):
#   zt/yt        d on partitions, 4 chunks of 128; DMA issue cost (~0.6us per
#                dma_start on the issuing sequencer) is spread over the three
#                DMA-capable engines (sync/scalar HWDGE + gpsimd SWDGE)
#   fused proj   stationary [wk|wv] (even blocks) / [wv|wk] (odd): one pass
#                over z yields K^T and V^T together; [wq|wq] duplicates Q^T
#                across both partition halves for free
#   K^T          kt_blk[j] [128,512]: tiles 8j..8j+3 on partitions 0:64,
#                tiles 8j+4..8j+7 on 64:128 -> 2-way row-packed S matmuls
#   V            V^T transposed tile-wise on the PE (+ones column appended)
#   S^T pair     two concurrent row-group matmuls (tiles 8j+i, 8j+4+i)
#   E            = exp(S^T * 0.125)  (ScalarE, PSUM -> SBUF bf16)
#   O^T[65,n]   += V_aug.T @ E       (row 64 = softmax denominator)
#   O[n,64]      = transpose(O^T) * (1/denom) + bv  -> DRAM
#
# bq/bk fold into the PSUM->SBUF projection copies as per-partition bias; bv
# is added at the end (softmax rows sum to 1 so P @ (1 bv^T) == 1 bv^T).

import numpy as np
import ml_dtypes

B = 4
M = 4096
N = 4096
D = 512
KQ = 64
NH = N // 2          # per-core query rows
NCORES = 8
SCALE = 0.125        # 1/sqrt(64)

NT = M // 128        # 32 m-tiles
NPAIR = NT // 2      # 16 pairs (8j+i, 8j+4+i)
NBLK = 4             # n-blocks of 512 in the attention loop
QBLK = NH // 512     # 4 q-projection blocks
KBLK = M // 512      # 8 fused kv-projection blocks

_CACHE = {}


def build_program():
    from contextlib import ExitStack

    import concourse.bacc as bacc
    import concourse.mybir as mybir
    import concourse.tile as tile
    from concourse.bass import ts, ds
    from concourse.masks import make_identity

    F32 = mybir.dt.float32
    BF16 = mybir.dt.bfloat16
    EXP = mybir.ActivationFunctionType.Exp

    nc = bacc.Bacc("TRN2", target_bir_lowering=False, debug=False)

    zt_d = nc.dram_tensor("zt", [4, 128, M], BF16, kind="ExternalInput").ap()
    yt_d = nc.dram_tensor("yt", [4, 128, NH], BF16, kind="ExternalInput").ap()
    # 12 fused weight tiles: [wq|wq]x4, [wk|wv]x4, [wv|wk]x4
    wp_d = nc.dram_tensor("wpack", [128, 1536], BF16, kind="ExternalInput").ap()
    # col 0: bq (dup both halves), col 1: bk (dup)
    bp_d = nc.dram_tensor("bpack", [128, 2], F32, kind="ExternalInput").ap()
    o_d = nc.dram_tensor("o", [KQ + 1, NH], F32, kind="ExternalOutput").ap()

    with ExitStack() as ctx:
        tc = ctx.enter_context(tile.TileContext(nc))
        singles = ctx.enter_context(tc.tile_pool(name="singles", bufs=1))
        epool = ctx.enter_context(tc.tile_pool(name="epool", bufs=4))
        vtpool = ctx.enter_context(tc.tile_pool(name="vtpool", bufs=2))
        otpool = ctx.enter_context(tc.tile_pool(name="otpool", bufs=2))
        spool = ctx.enter_context(tc.tile_pool(name="spool", bufs=2, space="PSUM"))
        opool = ctx.enter_context(tc.tile_pool(name="opool", bufs=1, space="PSUM"))
        ppool = ctx.enter_context(tc.tile_pool(name="ppool", bufs=2, space="PSUM"))

        # --- constants ---
        wpack = singles.tile([128, 1536], BF16, name="wpack", tag="wpack")
        nc.sync.dma_start(wpack[:, 0:512], wp_d[:, 0:512])
        nc.scalar.dma_start(wpack[:, 512:1024], wp_d[:, 512:1024])
        nc.gpsimd.dma_start(wpack[:, 1024:1536], wp_d[:, 1024:1536])
        bpack = singles.tile([128, 2], F32, name="bpack", tag="bpack")
        nc.sync.dma_start(bpack, bp_d)
        wqq = [wpack[:, ts(c, 128)] for c in range(4)]
        wkv_e = [wpack[:, ds(512 + c * 128, 128)] for c in range(4)]
        wkv_o = [wpack[:, ds(1024 + c * 128, 128)] for c in range(4)]
        bq_sb = bpack[:, 0:1]
        bk_sb = bpack[:, 1:2]

        identb = singles.tile([128, 128], BF16, name="identb", tag="identb")
        make_identity(nc, identb)

        # warm the exp table while DMAs stream
        warm = singles.tile([64, 1], F32, name="warm", tag="warm")
        nc.scalar.activation(warm, bpack[0:64, 0:1], EXP, scale=1.0)

        # --- activation SBUF tensors (filled by the DMAs below) ---
        yt = [
            singles.tile([128, NH], BF16, name=f"yt{c}", tag=f"yt{c}")
            for c in range(4)
        ]
        zt = [
            [
                singles.tile([128, 2048], BF16, name=f"zt{c}{h}", tag=f"zt{c}{h}")
                for h in range(2)
            ]
            for c in range(4)
        ]
        qt_blk = [
            singles.tile([128, 512], BF16, name=f"qt{j}", tag=f"qt{j}")
            for j in range(QBLK)
        ]
        kt_blk = [
            singles.tile([128, 512], BF16, name=f"kt{j}", tag=f"kt{j}")
            for j in range(4)
        ]
        v_sb = [
            singles.tile([128, KQ + 1], BF16, name=f"v{t}", tag=f"v{t}")
            for t in range(NT)
        ]

        # --- all input DMAs issued upfront, spread over 3 issuing engines
        # (each engine owns one DMA queue; ~95-128 GB/s per queue). y piece 0
        # gates q_proj(0) and with it every S matmul, so it goes first on
        # every queue; z blocks follow in consumption order. ---
        engs = [nc.sync, nc.scalar, nc.gpsimd]

        def dma_y_piece(pc):
            for c in range(4):
                engs[c % 3].dma_start(
                    yt[c][:, ds(pc * 1024, 1024)],
                    yt_d[c, :, ds(pc * 1024, 1024)],
                )

        def dma_z_piece(h, pc):
            for c in range(4):
                engs[(c + h + pc) % 3].dma_start(
                    zt[c][h][:, ds(pc * 1024, 1024)],
                    zt_d[c, :, ds(h * 2048 + pc * 1024, 1024)],
                )

        dma_y_piece(0)
        dma_z_piece(0, 0)
        dma_z_piece(0, 1)
        dma_y_piece(1)
        dma_z_piece(1, 0)
        dma_z_piece(1, 1)

        IDENT_FN = mybir.ActivationFunctionType.Identity

        def q_proj(j, on_act=False):
            # [wq|wq] stationary -> Q^T appears on both partition halves
            q_ps = ppool.tile([128, 512], F32, name="proj", tag="proj")
            for c in range(4):
                nc.tensor.matmul(
                    q_ps,
                    lhsT=wqq[c],
                    rhs=yt[c][:, ts(j, 512)],
                    start=(c == 0),
                    stop=(c == 3),
                )
            if on_act:
                nc.scalar.activation(qt_blk[j], q_ps, IDENT_FN, bias=bq_sb)
            else:
                nc.vector.tensor_scalar_add(qt_blk[j], q_ps, bq_sb)

        def kv_mm(b, on_act=False):
            # one pass over z m-block b (tiles 4b..4b+3) produces
            # K^T -> kt_blk[b//2] (even b: partitions 0:64, odd: 64:128)
            # and V^T -> staging tile (returned for kv_trans)
            h = b // 4
            bb = b % 4
            odd = b % 2
            w = wkv_o if odd else wkv_e
            khalf = slice(64, 128) if odd else slice(0, 64)
            vhalf = slice(0, 64) if odd else slice(64, 128)
            kv_ps = ppool.tile([128, 512], F32, name="proj", tag="proj")
            for c in range(4):
                nc.tensor.matmul(
                    kv_ps,
                    lhsT=w[c],
                    rhs=zt[c][h][:, ts(bb, 512)],
                    start=(c == 0),
                    stop=(c == 3),
                )
            if on_act:
                nc.scalar.activation(
                    kt_blk[b // 2][khalf, :], kv_ps[khalf, :], IDENT_FN,
                    bias=bk_sb[khalf, :],
                )
            else:
                nc.vector.tensor_scalar_add(
                    kt_blk[b // 2][khalf, :], kv_ps[khalf, :], bk_sb[khalf, :]
                )
            vt_sb = vtpool.tile([128, 512], BF16, name="vt", tag="vt")
            nc.vector.tensor_copy(vt_sb[vhalf, :], kv_ps[vhalf, :])
            return vt_sb, vhalf

        def kv_trans(b, vt_sb, vhalf):
            # V^T staging -> PE transpose -> natural-layout V tiles (+ones col)
            odd = b % 2
            vib = identb[64:128, 64:128] if odd == 0 else identb[0:64, 0:64]
            for i in range(4):
                t = 4 * b + i
                v_ps = ppool.tile([128, 512], BF16, name="projb", tag="proj")
                nc.tensor.matmul(
                    v_ps[:, 0:KQ],
                    lhsT=vt_sb[vhalf, ts(i, 128)],
                    rhs=vib,
                    is_transpose=True,
                    start=True,
                    stop=True,
                    tile_position=(vhalf.start, 0),
                )
                nc.vector.tensor_copy(v_sb[t][:, 0:KQ], v_ps[:, 0:KQ])
                nc.vector.memset(v_sb[t][:, KQ : KQ + 1], 1.0)

        def finalize(nb, o_ps):
            # ship unnormalized O^T (+denominator row 64) out; the host does
            # the cheap per-row divide and bias add during unsharding
            ot_sb = otpool.tile([128, 512], F32, name="ot", tag="ot")
            nc.vector.tensor_copy(ot_sb[0:65, :], o_ps[0:65, :])
            nc.sync.dma_start(o_d[:, ds(nb * 512, 512)], ot_sb[0:65, :])

        # --- PE warm-up spin: ~5us of dummy back-to-back matmuls while the
        # first input DMAs stream, so the HAM clock-gate opens (2.4 GHz)
        # before the projection chain begins ---
        wu_ps = opool.tile([128, 1024], F32, name="o", tag="o")
        for _ in range(90):
            nc.tensor.matmul(
                wu_ps[:, 0:128],
                lhsT=identb,
                rhs=identb,
                start=True,
                stop=True,
            )

        s_tiles = {}

        def s_mms(nb, p):
            jj, col = divmod(p, 4)
            s_ps = spool.tile([128, 1024], F32, name="s", tag="s")
            nc.tensor.matmul(
                s_ps[:, 0:512],
                lhsT=kt_blk[jj][0:64, ts(col, 128)],
                rhs=qt_blk[nb][0:64, :],
                start=True,
                stop=True,
                tile_position=(0, 0),
            )
            nc.tensor.matmul(
                s_ps[:, 512:1024],
                lhsT=kt_blk[jj][64:128, ts(col, 128)],
                rhs=qt_blk[nb][64:128, :],
                start=True,
                stop=True,
                tile_position=(64, 0),
            )
            s_tiles[(nb, p)] = s_ps

        def exp_pv(nb, p, o_acc):
            jj, col = divmod(p, 4)
            s_ps = s_tiles.pop((nb, p))
            e_t = epool.tile([128, 1024], BF16, name="e", tag="e")
            nc.scalar.activation(e_t, s_ps, EXP, scale=SCALE)
            osl = o_acc[0:65, ds((nb % 2) * 512, 512)]
            nc.tensor.matmul(
                osl,
                lhsT=v_sb[8 * jj + col],
                rhs=e_t[:, 0:512],
                start=(p == 0),
                stop=False,
            )
            nc.tensor.matmul(
                osl,
                lhsT=v_sb[8 * jj + 4 + col],
                rhs=e_t[:, 512:1024],
                start=False,
                stop=(p == NPAIR - 1),
            )

        def finalize(nb, o_acc):
            # ship unnormalized O^T (+denominator row 64) out; the host does
            # the cheap per-row divide and bias add during unsharding
            ot_sb = otpool.tile([128, 512], F32, name="ot", tag="ot")
            nc.vector.tensor_copy(ot_sb[0:65, :], o_acc[0:65, ds((nb % 2) * 512, 512)])
            nc.sync.dma_start(o_d[:, ds(nb * 512, 512)], ot_sb[0:65, :])

        # --- two-pass group-major attention: pass 1 covers n-blocks {0,1}
        # while z streams in (pair group g only needs z m-blocks 2g, 2g+1);
        # pass 2 covers n-blocks {2,3} entirely from SBUF ---
        q_proj(0, on_act=True)
        q_proj(1, on_act=True)
        o_acc1 = opool.tile([128, 1024], F32, name="o", tag="o")
        for g in range(4):
            vt_a = kv_mm(2 * g, on_act=(g == 0))
            vt_b = kv_mm(2 * g + 1, on_act=(g == 0))
            seq = [(nb, 4 * g + i) for i in range(4) for nb in (0, 1)]
            s_mms(*seq[0])
            s_mms(*seq[1])
            kv_trans(2 * g, *vt_a)
            kv_trans(2 * g + 1, *vt_b)
            for k in range(8):
                exp_pv(*seq[k], o_acc1)
                if k + 2 < 8:
                    s_mms(*seq[k + 2])
            if g == 1:
                q_proj(2)
            if g == 2:
                q_proj(3)
        finalize(0, o_acc1)
        finalize(1, o_acc1)

        o_acc2 = opool.tile([128, 1024], F32, name="o", tag="o")
        seq = [(nb, p) for p in range(NPAIR) for nb in (2, 3)]
        s_mms(*seq[0])
        s_mms(*seq[1])
        for k in range(len(seq)):
            exp_pv(*seq[k], o_acc2)
            if k + 2 < len(seq):
                s_mms(*seq[k + 2])
        finalize(2, o_acc2)
        finalize(3, o_acc2)

    nc.compile()
    return nc


def _get_program():
    if "nc" not in _CACHE:
        _CACHE["nc"] = build_program()
    return _CACHE["nc"]


def make_in_maps(z, y, Wq, bq, Wk, bk, Wv, bv):
    bf16 = ml_dtypes.bfloat16
    zt = np.ascontiguousarray(z.astype(bf16).transpose(0, 2, 1))  # [B, 512, M]
    yt = np.ascontiguousarray(y.astype(bf16).transpose(0, 2, 1))  # [B, 512, N]
    wq = Wq.astype(bf16).reshape(4, 128, KQ)
    wk = Wk.astype(bf16).reshape(4, 128, KQ)
    wv = Wv.astype(bf16).reshape(4, 128, KQ)
    wpack = np.empty((128, 1536), dtype=bf16)
    for c in range(4):
        wpack[:, c * 128 : c * 128 + 64] = wq[c]
        wpack[:, c * 128 + 64 : c * 128 + 128] = wq[c]
        wpack[:, 512 + c * 128 : 512 + c * 128 + 64] = wk[c]
        wpack[:, 512 + c * 128 + 64 : 512 + c * 128 + 128] = wv[c]
        wpack[:, 1024 + c * 128 : 1024 + c * 128 + 64] = wv[c]
        wpack[:, 1024 + c * 128 + 64 : 1024 + c * 128 + 128] = wk[c]
    bpack = np.empty((128, 2), dtype=np.float32)
    bpack[0:64, 0] = bq
    bpack[64:128, 0] = bq
    bpack[0:64, 1] = bk
    bpack[64:128, 1] = bk
    in_maps = []
    for c in range(NCORES):
        b, h = divmod(c, 2)
        in_maps.append(
            {
                "zt": zt[b].reshape(4, 128, M),
                "yt": np.ascontiguousarray(
                    yt[b][:, h * NH : (h + 1) * NH]
                ).reshape(4, 128, NH),
                "wpack": wpack,
                "bpack": bpack,
            }
        )
    return in_maps


def kernel(z, y, Wq, bq, Wk, bk, Wv, bv):
    from concourse import bass_utils

    nc = _get_program()
    in_maps = make_in_maps(z, y, Wq, bq, Wk, bk, Wv, bv)
    res = bass_utils.run_bass_kernel_spmd(nc, in_maps, core_ids=list(range(NCORES)))
    return assemble_output(res.results, bv)


def assemble_output(results, bv):
    out = np.empty((B, N, KQ), dtype=np.float32)
    bvf = bv.astype(np.float32)[None, :]
    for c in range(NCORES):
        b, h = divmod(c, 2)
        ot = results[c]["o"]  # [65, NH]: rows 0:64 = O^T unnorm, row 64 = denom
        out[b, h * NH : (h + 1) * NH, :] = (ot[0:KQ] / ot[KQ : KQ + 1]).T + bvf
    return out


# revision 15
# speedup vs baseline: 1.1861x; 1.0899x over previous
# Cross-attention SDPA kernel for 8 Trainium2 NeuronCores.
#
# reference semantics (per batch b):
#   Q = y @ Wq + bq            [N, 64]
#   K = z @ Wk + bk            [M, 64]
#   V = z @ Wv + bv            [M, 64]
#   O = softmax(Q K^T / 8) V   [N, 64]
# B=4, M=N=4096, D=512.
#
# Sharding: 8 cores = 4 batches x 2 halves of the query (decoder) length.
# Each core gets z^T[b] (full, [512,4096]) and its y^T half ([512,2048]),
# pre-transposed and cast to bf16 on the host, and produces O rows
# [2048, 64] fp32.
#
# On-core dataflow (S^T layout so the softmax reduction rides the matmul):
#   zt/yt        d on partitions, 4 chunks of 128; DMA issue cost (~0.6us per
#                dma_start on the issuing sequencer) is spread over the three
#                DMA-capable engines (sync/scalar HWDGE + gpsimd SWDGE)
#   fused proj   stationary [wk|wv] (even blocks) / [wv|wk] (odd): one pass
#                over z yields K^T and V^T together; [wq|wq] duplicates Q^T
#                across both partition halves for free
#   K^T          kt_blk[j] [128,512]: tiles 8j..8j+3 on partitions 0:64,
#                tiles 8j+4..8j+7 on 64:128 -> 2-way row-packed S matmuls
#   V            V^T transposed tile-wise on the PE (+ones column appended)
#   S^T pair     two concurrent row-group matmuls (tiles 8j+i, 8j+4+i)
#   E            = exp(S^T * 0.125)  (ScalarE, PSUM -> SBUF bf16)
#   O^T[65,n]   += V_aug.T @ E       (row 64 = softmax denominator)
#   O[n,64]      = transpose(O^T) * (1/denom) + bv  -> DRAM
#
# bq/bk fold into the PSUM->SBUF projection copies as per-partition bias; bv
# is added at the end (softmax rows sum to 1 so P @ (1 bv^T) == 1 bv^T).

import numpy as np
import ml_dtypes

B = 4
M = 4096
N = 4096
D = 512
KQ = 64
NH = N // 2          # per-core query rows
NCORES = 8
SCALE = 0.125        # 1/sqrt(64)

NT = M // 128        # 32 m-tiles
NPAIR = NT // 2      # 16 pairs (8j+i, 8j+4+i)
NBLK = 4             # n-blocks of 512 in the attention loop
QBLK = NH // 512     # 4 q-projection blocks
KBLK = M // 512      # 8 fused kv-projection blocks

_CACHE = {}


def build_program():
    from contextlib import ExitStack

    import concourse.bacc as bacc
    import concourse.mybir as mybir
    import concourse.tile as tile
    from concourse.bass import ts, ds
    from concourse.masks import make_identity

    F32 = mybir.dt.float32
    BF16 = mybir.dt.bfloat16
    EXP = mybir.ActivationFunctionType.Exp

    nc = bacc.Bacc("TRN2", target_bir_lowering=False, debug=False)

    zt_d = nc.dram_tensor("zt", [4, 128, M], BF16, kind="ExternalInput").ap()
    yt_d = nc.dram_tensor("yt", [4, 128, NH], BF16, kind="ExternalInput").ap()
    # 12 fused weight tiles: [wq|wq]x4, [wk|wv]x4, [wv|wk]x4
    wp_d = nc.dram_tensor("wpack", [128, 1536], BF16, kind="ExternalInput").ap()
    # col 0: bq (dup both halves), col 1: bk (dup)
    bp_d = nc.dram_tensor("bpack", [128, 2], F32, kind="ExternalInput").ap()
    o_d = nc.dram_tensor("o", [KQ + 1, NH], F32, kind="ExternalOutput").ap()

    with ExitStack() as ctx:
        tc = ctx.enter_context(tile.TileContext(nc))
        singles = ctx.enter_context(tc.tile_pool(name="singles", bufs=1))
        epool = ctx.enter_context(tc.tile_pool(name="epool", bufs=4))
        vtpool = ctx.enter_context(tc.tile_pool(name="vtpool", bufs=2))
        otpool = ctx.enter_context(tc.tile_pool(name="otpool", bufs=2))
        spool = ctx.enter_context(tc.tile_pool(name="spool", bufs=2, space="PSUM"))
        opool = ctx.enter_context(tc.tile_pool(name="opool", bufs=1, space="PSUM"))
        ppool = ctx.enter_context(tc.tile_pool(name="ppool", bufs=2, space="PSUM"))

        # --- constants ---
        bpack = singles.tile([128, 2], F32, name="bpack", tag="bpack")
        nc.sync.dma_start(bpack, bp_d)
        wpack = singles.tile([128, 1536], BF16, name="wpack", tag="wpack")
        nc.sync.dma_start(wpack[:, 1024:1536], wp_d[:, 1024:1536])
        nc.gpsimd.dma_start(wpack[:, 0:1024], wp_d[:, 0:1024])
        wqq = [wpack[:, ts(c, 128)] for c in range(4)]
        wkv_e = [wpack[:, ds(512 + c * 128, 128)] for c in range(4)]
        wkv_o = [wpack[:, ds(1024 + c * 128, 128)] for c in range(4)]
        bq_sb = bpack[:, 0:1]
        bk_sb = bpack[:, 1:2]

        identb = singles.tile([128, 128], BF16, name="identb", tag="identb")
        make_identity(nc, identb)

        # warm the exp table while DMAs stream
        warm = singles.tile([64, 1], F32, name="warm", tag="warm")
        nc.scalar.activation(warm, bpack[0:64, 0:1], EXP, scale=1.0)

        # --- activation SBUF tensors (filled by the DMAs below) ---
        yt = [
            [
                singles.tile([128, 1024], BF16, name=f"yt{c}{j}", tag=f"yt{c}{j}")
                for j in range(2)
            ]
            for c in range(4)
        ]
        zt = [
            [
                [
                    singles.tile(
                        [128, 1024], BF16, name=f"zt{c}{h}{pc}", tag=f"zt{c}{h}{pc}"
                    )
                    for pc in range(2)
                ]
                for h in range(2)
            ]
            for c in range(4)
        ]
        qt_blk = [
            singles.tile([128, 512], BF16, name=f"qt{j}", tag=f"qt{j}")
            for j in range(QBLK)
        ]
        kt_blk = [
            singles.tile([128, 512], BF16, name=f"kt{j}", tag=f"kt{j}")
            for j in range(4)
        ]
        v_sb = [
            singles.tile([128, KQ + 1], BF16, name=f"v{t}", tag=f"v{t}")
            for t in range(NT)
        ]

        # --- all input DMAs issued upfront. sync + gpsimd own the bulk (one
        # DMA queue each, ~95-128 GB/s); scalar only issues two early pieces
        # that complete before the exp stream starts, so DMA ring
        # flow-control never blocks the ACT instruction stream. ---
        def dma_y_piece(pc, engs):
            for c in range(4):
                engs[c].dma_start(
                    yt[c][pc], yt_d[c, :, ds(pc * 1024, 1024)]
                )

        def dma_z_piece(h, pc, engs):
            for c in range(4):
                engs[c].dma_start(
                    zt[c][h][pc], zt_d[c, :, ds(h * 2048 + pc * 1024, 1024)]
                )

        gp, sy, sc = nc.gpsimd, nc.sync, nc.scalar
        dma_y_piece(0, [gp, sy, gp, sc])
        dma_z_piece(0, 0, [sy, gp, sc, sy])
        dma_z_piece(0, 1, [gp, sy, gp, sy])
        dma_y_piece(1, [gp, sy, gp, sy])
        dma_z_piece(1, 0, [gp, sy, gp, sy])
        dma_z_piece(1, 1, [gp, sy, gp, sy])

        IDENT_FN = mybir.ActivationFunctionType.Identity

        def q_proj(j, on_act=False):
            # [wq|wq] stationary -> Q^T appears on both partition halves
            q_ps = ppool.tile([128, 512], F32, name="proj", tag="proj")
            for c in range(4):
                nc.tensor.matmul(
                    q_ps,
                    lhsT=wqq[c],
                    rhs=yt[c][j // 2][:, ts(j % 2, 512)],
                    start=(c == 0),
                    stop=(c == 3),
                )
            if on_act:
                nc.scalar.activation(qt_blk[j], q_ps, IDENT_FN, bias=bq_sb)
            else:
                nc.vector.tensor_scalar_add(qt_blk[j], q_ps, bq_sb)

        def kv_mm(b, on_act=False):
            # one pass over z m-block b (tiles 4b..4b+3) produces
            # K^T -> kt_blk[b//2] (even b: partitions 0:64, odd: 64:128)
            # and V^T -> staging tile (returned for kv_trans)
            h = b // 4
            pc, bb = divmod(b % 4, 2)
            odd = b % 2
            w = wkv_o if odd else wkv_e
            khalf = slice(64, 128) if odd else slice(0, 64)
            vhalf = slice(0, 64) if odd else slice(64, 128)
            kv_ps = ppool.tile([128, 512], F32, name="proj", tag="proj")
            for c in range(4):
                nc.tensor.matmul(
                    kv_ps,
                    lhsT=w[c],
                    rhs=zt[c][h][pc][:, ts(bb, 512)],
                    start=(c == 0),
                    stop=(c == 3),
                )
            if on_act:
                nc.scalar.activation(
                    kt_blk[b // 2][khalf, :], kv_ps[khalf, :], IDENT_FN,
                    bias=bk_sb[khalf, :],
                )
            else:
                nc.vector.tensor_scalar_add(
                    kt_blk[b // 2][khalf, :], kv_ps[khalf, :], bk_sb[khalf, :]
                )
            vt_sb = vtpool.tile([128, 512], BF16, name="vt", tag="vt")
            nc.vector.tensor_copy(vt_sb[vhalf, :], kv_ps[vhalf, :])
            return vt_sb, vhalf

        def kv_trans(b, vt_sb, vhalf):
            # V^T staging -> PE transpose -> natural-layout V tiles (+ones col)
            odd = b % 2
            vib = identb[64:128, 64:128] if odd == 0 else identb[0:64, 0:64]
            for i in range(4):
                t = 4 * b + i
                v_ps = ppool.tile([128, 512], BF16, name="projb", tag="proj")
                nc.tensor.matmul(
                    v_ps[:, 0:KQ],
                    lhsT=vt_sb[vhalf, ts(i, 128)],
                    rhs=vib,
                    is_transpose=True,
                    start=True,
                    stop=True,
                    tile_position=(vhalf.start, 0),
                )
                nc.vector.tensor_copy(v_sb[t][:, 0:KQ], v_ps[:, 0:KQ])
                nc.vector.memset(v_sb[t][:, KQ : KQ + 1], 1.0)

        def finalize(nb, o_ps):
            # ship unnormalized O^T (+denominator row 64) out; the host does
            # the cheap per-row divide and bias add during unsharding
            ot_sb = otpool.tile([128, 512], F32, name="ot", tag="ot")
            nc.vector.tensor_copy(ot_sb[0:65, :], o_ps[0:65, :])
            nc.sync.dma_start(o_d[:, ds(nb * 512, 512)], ot_sb[0:65, :])

        s_tiles = {}

        def s_mms(nb, p):
            jj, col = divmod(p, 4)
            s_ps = spool.tile([128, 1024], F32, name="s", tag="s")
            nc.tensor.matmul(
                s_ps[:, 0:512],
                lhsT=kt_blk[jj][0:64, ts(col, 128)],
                rhs=qt_blk[nb][0:64, :],
                start=True,
                stop=True,
                tile_position=(0, 0),
            )
            nc.tensor.matmul(
                s_ps[:, 512:1024],
                lhsT=kt_blk[jj][64:128, ts(col, 128)],
                rhs=qt_blk[nb][64:128, :],
                start=True,
                stop=True,
                tile_position=(64, 0),
            )
            s_tiles[(nb, p)] = s_ps

        def exp_pv(nb, p, o_acc):
            jj, col = divmod(p, 4)
            s_ps = s_tiles.pop((nb, p))
            e_t = epool.tile([128, 1024], BF16, name="e", tag="e")
            nc.scalar.activation(e_t, s_ps, EXP, scale=SCALE)
            osl = o_acc[0:65, ds((nb % 2) * 512, 512)]
            nc.tensor.matmul(
                osl,
                lhsT=v_sb[8 * jj + col],
                rhs=e_t[:, 0:512],
                start=(p == 0),
                stop=False,
            )
            nc.tensor.matmul(
                osl,
                lhsT=v_sb[8 * jj + 4 + col],
                rhs=e_t[:, 512:1024],
                start=False,
                stop=(p == NPAIR - 1),
            )

        def finalize(nb, o_acc):
            # ship unnormalized O^T (+denominator row 64) out; the host does
            # the cheap per-row divide and bias add during unsharding
            ot_sb = otpool.tile([128, 512], F32, name="ot", tag="ot")
            nc.vector.tensor_copy(ot_sb[0:65, :], o_acc[0:65, ds((nb % 2) * 512, 512)])
            nc.sync.dma_start(o_d[:, ds(nb * 512, 512)], ot_sb[0:65, :])

        # --- two-pass group-major attention: pass 1 covers n-blocks {0,1}
        # while z streams in (pair group g only needs z m-blocks 2g, 2g+1);
        # pass 2 covers n-blocks {2,3} entirely from SBUF ---
        q_proj(0, on_act=True)
        q_proj(1, on_act=True)
        o_acc1 = opool.tile([128, 1024], F32, name="o", tag="o")
        for g in range(4):
            vt_a = kv_mm(2 * g, on_act=(g == 0))
            vt_b = kv_mm(2 * g + 1, on_act=(g == 0))
            seq = [(nb, 4 * g + i) for i in range(4) for nb in (0, 1)]
            s_mms(*seq[0])
            s_mms(*seq[1])
            kv_trans(2 * g, *vt_a)
            kv_trans(2 * g + 1, *vt_b)
            for k in range(8):
                exp_pv(*seq[k], o_acc1)
                if k + 2 < 8:
                    s_mms(*seq[k + 2])
            if g == 1:
                q_proj(2)
            if g == 2:
                q_proj(3)
        finalize(0, o_acc1)
        finalize(1, o_acc1)

        o_acc2 = opool.tile([128, 1024], F32, name="o", tag="o")
        seq = [(nb, p) for p in range(NPAIR) for nb in (2, 3)]
        s_mms(*seq[0])
        s_mms(*seq[1])
        for k in range(len(seq)):
            exp_pv(*seq[k], o_acc2)
            if k + 2 < len(seq):
                s_mms(*seq[k + 2])
        finalize(2, o_acc2)
        finalize(3, o_acc2)

    nc.compile()
    return nc


def _get_program():
    if "nc" not in _CACHE:
        _CACHE["nc"] = build_program()
    return _CACHE["nc"]


def make_in_maps(z, y, Wq, bq, Wk, bk, Wv, bv):
    bf16 = ml_dtypes.bfloat16
    zt = np.ascontiguousarray(z.astype(bf16).transpose(0, 2, 1))  # [B, 512, M]
    yt = np.ascontiguousarray(y.astype(bf16).transpose(0, 2, 1))  # [B, 512, N]
    wq = Wq.astype(bf16).reshape(4, 128, KQ)
    wk = Wk.astype(bf16).reshape(4, 128, KQ)
    wv = Wv.astype(bf16).reshape(4, 128, KQ)
    wpack = np.empty((128, 1536), dtype=bf16)
    for c in range(4):
        wpack[:, c * 128 : c * 128 + 64] = wq[c]
        wpack[:, c * 128 + 64 : c * 128 + 128] = wq[c]
        wpack[:, 512 + c * 128 : 512 + c * 128 + 64] = wk[c]
        wpack[:, 512 + c * 128 + 64 : 512 + c * 128 + 128] = wv[c]
        wpack[:, 1024 + c * 128 : 1024 + c * 128 + 64] = wv[c]
        wpack[:, 1024 + c * 128 + 64 : 1024 + c * 128 + 128] = wk[c]
    bpack = np.empty((128, 2), dtype=np.float32)
    bpack[0:64, 0] = bq
    bpack[64:128, 0] = bq
    bpack[0:64, 1] = bk
    bpack[64:128, 1] = bk
    in_maps = []
    for c in range(NCORES):
        b, h = divmod(c, 2)
        in_maps.append(
            {
                "zt": zt[b].reshape(4, 128, M),
                "yt": np.ascontiguousarray(
                    yt[b][:, h * NH : (h + 1) * NH]
                ).reshape(4, 128, NH),
                "wpack": wpack,
                "bpack": bpack,
            }
        )
    return in_maps


def kernel(z, y, Wq, bq, Wk, bk, Wv, bv):
    from concourse import bass_utils

    nc = _get_program()
    in_maps = make_in_maps(z, y, Wq, bq, Wk, bk, Wv, bv)
    res = bass_utils.run_bass_kernel_spmd(nc, in_maps, core_ids=list(range(NCORES)))
    return assemble_output(res.results, bv)


def assemble_output(results, bv):
    out = np.empty((B, N, KQ), dtype=np.float32)
    bvf = bv.astype(np.float32)[None, :]
    for c in range(NCORES):
        b, h = divmod(c, 2)
        ot = results[c]["o"]  # [65, NH]: rows 0:64 = O^T unnorm, row 64 = denom
        out[b, h * NH : (h + 1) * NH, :] = (ot[0:KQ] / ot[KQ : KQ + 1]).T + bvf
    return out
